# revision 1
# baseline (speedup 1.0000x reference)
"""GCN layer (BN -> dense -> sparse softmax -> gather/scatter -> tanh) on 8
Trainium2 NeuronCores.

Strategy (1D edge parallelism, gather-free):
 - Destination nodes are sharded 12500/core; each edge lives on the core that
   owns its destination row. The host materializes each edge slot's SOURCE
   features (x_exp[slot] = x[col], fp16) as part of edge sharding, so the
   device needs no data-dependent addressing at all (the per-edge gather was
   Q7-descriptor-bound at ~8 ns/edge).
 - Per core, edges are laid out per 128-destination-node window, padded to
   kw 128-edge chunks. Per chunk ONE PE matmul does gather+scatter+softmax
   denominator at once:  A_win[i, 0:128] += M^T @ (x_exp * exp(v)),
   A_win[i, 128] += M^T @ exp(v), with M[e, i] = (loc[e] == i) a one-hot
   matrix built on the vector engine via iota-compare.
 - BatchNorm folds into the projection: per-core partial sums -> AllReduce
   (the only collective) -> W' = rstd*W, b' = -mean*rstd @ W'. Per window:
   out = tanh((A[:, :128] @ W') / den + b'), zeroed for edgeless nodes.
 - Softmax needs no max subtraction: edge_vals are uniform [0,1).

Numerics: matmul operands fp16 (PSUM accumulates fp32); stats, softmax
denominator and the flush in fp32.
"""
import sys

sys.path.insert(0, "/opt/trn_rl_repo")

import numpy as np
from contextlib import ExitStack

import concourse.bass as bass
import concourse.bacc as bacc
import concourse.mybir as mybir
import concourse.tile as tile
from concourse.bass_utils import run_bass_kernel_spmd

# problem constants
N = 100000
E = 1600000
F = 128
D = 64
BN_EPS = 1e-3
NCORES = 8
NPC = N // NCORES            # 12500 destination nodes per core
WIN = 128                    # destination nodes per window
NW = (NPC + WIN - 1) // WIN  # 98 windows per core (last window 84 nodes)

f16, f32 = mybir.dt.float16, mybir.dt.float32

_cache: dict[int, object] = {}


def _group_sizes():
    gs, w = [], NW
    while w > 0:
        g = min(3, w)
        gs.append(g)
        w -= g
    return gs


def _build(kw: int):
    """Build the SPMD program. kw = max 128-edge chunks per window."""
    nch = NW * kw                    # chunks per core

    nc = bacc.Bacc(None, target_bir_lowering=False)

    xT = nc.declare_dram_parameter("xT", [F, NPC], f16, isOutput=False)
    w_in = nc.declare_dram_parameter("w_in", [F, D], f32, isOutput=False)
    ident_in = nc.declare_dram_parameter("ident_in", [128, 128], f16, isOutput=False)
    meq_in = nc.declare_dram_parameter("meq_in", [128, nch * 128], f16, isOutput=False)
    val_in = nc.declare_dram_parameter("val_in", [128, nch], f32, isOutput=False)
    xe_in = nc.declare_dram_parameter("xe_in", [128, nch * F], f16, isOutput=False)
    out_p = nc.declare_dram_parameter("out", [NPC, D], f32, isOutput=True)

    with tile.TileContext(nc) as tc:
        with ExitStack() as ctx:
            sb = ctx.enter_context(tc.tile_pool(name="sb", bufs=1))
            pp = ctx.enter_context(tc.tile_pool(name="pp", bufs=1, space="PSUM"))
            dram = ctx.enter_context(tc.tile_pool(name="dram", bufs=1, space="DRAM"))

            # ---------------- phase 0: BN stats -> W', bias ----------------
            xts = sb.tile([F, NPC], f16)
            nc.sync.dma_start(out=xts[:], in_=xT[:])

            stats = sb.tile([F, 2], f32)
            nc.vector.tensor_reduce(
                out=stats[:, 0:1], in_=xts[:], axis=mybir.AxisListType.X,
                op=mybir.AluOpType.add)
            sq_trash = sb.tile([F, NPC], f16)
            nc.scalar.activation(
                out=sq_trash[:], in_=xts[:],
                func=mybir.ActivationFunctionType.Square,
                accum_out=stats[:, 1:2])

            st_b = dram.tile([F, 2], f32)
            red_b = dram.tile([F, 2], f32)
            nc.gpsimd.dma_start(out=st_b[:], in_=stats[:])
            nc.gpsimd.collective_compute(
                "AllReduce", mybir.AluOpType.add,
                replica_groups=[list(range(NCORES))],
                ins=[st_b[:].opt()], outs=[red_b[:].opt()])
            red = sb.tile([F, 2], f32)
            nc.gpsimd.dma_start(out=red[:], in_=red_b[:])

            mean = sb.tile([F, 1], f32)
            nc.vector.tensor_scalar_mul(out=mean[:], in0=red[:, 0:1], scalar1=1.0 / N)
            ex2 = sb.tile([F, 1], f32)
            nc.vector.tensor_scalar_mul(out=ex2[:], in0=red[:, 1:2], scalar1=1.0 / N)
            msq = sb.tile([F, 1], f32)
            nc.vector.tensor_tensor(out=msq[:], in0=mean[:], in1=mean[:],
                                    op=mybir.AluOpType.mult)
            varep = sb.tile([F, 1], f32)
            nc.vector.tensor_tensor(out=varep[:], in0=ex2[:], in1=msq[:],
                                    op=mybir.AluOpType.subtract)
            nc.vector.tensor_scalar_add(out=varep[:], in0=varep[:], scalar1=BN_EPS)
            sdev = sb.tile([F, 1], f32)
            nc.scalar.activation(out=sdev[:], in_=varep[:],
                                 func=mybir.ActivationFunctionType.Sqrt)
            rstd = sb.tile([F, 1], f32)
            nc.vector.reciprocal(out=rstd[:], in_=sdev[:])

            w_sb = sb.tile([F, D], f32)
            nc.sync.dma_start(out=w_sb[:], in_=w_in[:])
            wp = sb.tile([F, D], f16)
            nc.vector.tensor_scalar(out=wp[:], in0=w_sb[:], scalar1=rstd[:, 0:1],
                                    scalar2=None, op0=mybir.AluOpType.mult)
            nmr = sb.tile([F, 1], f32)
            nc.vector.tensor_tensor(out=nmr[:], in0=mean[:], in1=rstd[:],
                                    op=mybir.AluOpType.mult)
            nmr16 = sb.tile([F, 1], f16)
            nc.vector.tensor_scalar_mul(out=nmr16[:], in0=nmr[:], scalar1=-1.0)

            b_ps = pp.tile([128, D], f32, tag="init", bufs=2)
            nc.tensor.matmul(out=b_ps[:1, :], lhsT=nmr16[:], rhs=wp[:],
                             start=True, stop=True)
            b16 = sb.tile([1, D], f16)
            nc.vector.tensor_copy(out=b16[:], in_=b_ps[:1, :])
            ones_r = sb.tile([1, 128], f16)
            nc.vector.memset(ones_r[:], 1.0)
            bf_ps = pp.tile([128, D], f32, tag="init", bufs=2)
            nc.tensor.matmul(out=bf_ps[:], lhsT=ones_r[:], rhs=b16[:],
                             start=True, stop=True)
            bfull = sb.tile([128, D], f32)
            nc.vector.tensor_copy(out=bfull[:], in_=bf_ps[:])

            # ---------------- phase 1: edges ----------------
            val_sb = sb.tile([128, nch], f32)
            nc.sync.dma_start(out=val_sb[:], in_=val_in[:])
            ident_sb = sb.tile([128, 128], f16)
            nc.sync.dma_start(out=ident_sb[:], in_=ident_in[:])
            exp_sb = sb.tile([128, nch], f16)
            nc.scalar.activation(out=exp_sb[:], in_=val_sb[:],
                                 func=mybir.ActivationFunctionType.Exp)

            w0 = 0
            for gwn in _group_sizes():
                ch0 = w0 * kw
                gch = gwn * kw
                xw = sb.tile([128, gch, F], f16, tag="xw", bufs=2)
                nc.sync.dma_start(
                    out=xw[:], in_=xe_in[:, ch0 * F:(ch0 + gch) * F])
                mq = sb.tile([128, gch * 128], f16, tag="mq", bufs=2)
                nc.sync.dma_start(
                    out=mq[:], in_=meq_in[:, ch0 * 128:(ch0 + gch) * 128])
                xs = sb.tile([128, gch, F + 1], f16, tag="xs", bufs=2)
                nc.vector.tensor_tensor(
                    out=xs[:, :, 0:F], in0=xw[:],
                    in1=exp_sb[:, ch0:ch0 + gch].to_broadcast([128, gch, F]),
                    op=mybir.AluOpType.mult)
                nc.vector.tensor_copy(out=xs[:, :, F],
                                      in_=exp_sb[:, ch0:ch0 + gch])
                for wi in range(gwn):
                    w = w0 + wi
                    m = min(WIN, NPC - w * WIN)
                    A = pp.tile([128, F + 1], f32, tag="A", bufs=2)
                    for c in range(kw):
                        mof = (wi * kw + c) * 128
                        nc.tensor.matmul(
                            out=A[:], lhsT=mq[:, mof:mof + 128],
                            rhs=xs[:, wi * kw + c, :],
                            start=(c == 0), stop=(c == kw - 1))
                    As = sb.tile([128, 128], f16, tag="As", bufs=2)
                    nc.scalar.activation(out=As[:], in_=A[:, 0:F],
                                         func=mybir.ActivationFunctionType.Copy)
                    ATp = pp.tile([128, 128], f16, tag="ATp", bufs=2)
                    nc.tensor.transpose(out=ATp[:], in_=As[:], identity=ident_sb[:])
                    ATs = sb.tile([128, 128], f16, tag="ATs", bufs=2)
                    nc.scalar.activation(out=ATs[:], in_=ATp[:],
                                         func=mybir.ActivationFunctionType.Copy)
                    ps2 = pp.tile([128, D], f32, tag="ps2", bufs=2)
                    nc.tensor.matmul(out=ps2[:], lhsT=ATs[:], rhs=wp[:],
                                     start=True, stop=True)
                    # flush: out = tanh(num/den + b') masked to den>0
                    dmax = sb.tile([128, 1], f32, tag="dmax", bufs=4)
                    nc.vector.tensor_scalar_max(out=dmax[:], in0=A[:, F:F + 1],
                                                scalar1=1e-30)
                    ind = sb.tile([128, 1], f32, tag="ind", bufs=4)
                    nc.vector.tensor_scalar(out=ind[:], in0=A[:, F:F + 1],
                                            scalar1=0.0, scalar2=None,
                                            op0=mybir.AluOpType.is_gt)
                    rec = sb.tile([128, 1], f32, tag="rec", bufs=4)
                    nc.vector.reciprocal(out=rec[:], in_=dmax[:])
                    t1 = sb.tile([128, D], f32, tag="t1", bufs=4)
                    nc.vector.tensor_scalar(out=t1[:], in0=ps2[:],
                                            scalar1=rec[:, 0:1], scalar2=None,
                                            op0=mybir.AluOpType.mult)
                    t2 = sb.tile([128, D], f32, tag="t2", bufs=4)
                    nc.vector.tensor_tensor(out=t2[:], in0=t1[:], in1=bfull[:],
                                            op=mybir.AluOpType.add)
                    th = sb.tile([128, D], f32, tag="th", bufs=4)
                    nc.scalar.activation(out=th[:], in_=t2[:],
                                         func=mybir.ActivationFunctionType.Tanh)
                    ot = sb.tile([128, D], f32, tag="ot", bufs=4)
                    nc.vector.tensor_scalar(out=ot[:], in0=th[:],
                                            scalar1=ind[:, 0:1], scalar2=None,
                                            op0=mybir.AluOpType.mult)
                    nc.sync.dma_start(out=out_p[w * WIN:w * WIN + m, :],
                                      in_=ot[:m, :])
                w0 += gwn

    nc.finalize()
    return nc


def _prep(x, w, edge_vals, rows, cols, kw):
    """Host-side shard/layout construction. Returns in_maps or None if kw
    is too small for this edge distribution."""
    nch = NW * kw

    order = np.argsort(rows, kind="stable")
    rs = rows[order].astype(np.int64)
    cs = cols[order].astype(np.int64)
    vs = edge_vals[order]

    core = rs // NPC
    loc_in_core = rs % NPC
    w_in_core = loc_in_core // WIN
    loc = loc_in_core % WIN

    run = core * NW + w_in_core          # global window id, monotone in rs
    nruns = NCORES * NW
    counts = np.bincount(run, minlength=nruns)
    if counts.max() > kw * 128:
        return None
    starts = np.zeros(nruns, np.int64)
    np.cumsum(counts[:-1], out=starts[1:])
    pos = np.arange(len(run)) - starts[run]

    chunk = w_in_core * kw + pos // 128  # chunk index within the core
    e_part = pos % 128

    locf = np.full((NCORES, 128, nch), -1, np.int16)
    valf = np.full((NCORES, 128, nch), -100.0, np.float32)
    colf = np.zeros((NCORES, 128, nch), np.int64)
    locf[core, e_part, chunk] = loc.astype(np.int16)
    valf[core, e_part, chunk] = vs
    colf[core, e_part, chunk] = cs

    x16 = x.astype(np.float16)
    ident = np.eye(128, dtype=np.float16)
    rng128 = np.arange(128, dtype=np.int16)
    in_maps = []
    for c in range(NCORES):
        xe = np.ascontiguousarray(x16[colf[c]])          # [128, nch, F]
        meq = (locf[c][:, :, None] == rng128).astype(np.float16)
        xsh = np.ascontiguousarray(x16[c * NPC:(c + 1) * NPC, :].T)
        in_maps.append({
            "xT": xsh,
            "w_in": np.ascontiguousarray(w.astype(np.float32)),
            "ident_in": ident,
            "meq_in": meq.reshape(128, nch * 128),
            "val_in": np.ascontiguousarray(valf[c]),
            "xe_in": xe.reshape(128, nch * F),
        })
    return in_maps


def kernel(x, kernel, edge_vals, rows, cols, nodes_num):
    assert int(nodes_num) == N and x.shape == (N, F) and kernel.shape == (F, D)
    kw = 18
    in_maps = _prep(x, kernel, edge_vals, rows, cols, kw)
    while in_maps is None:  # pathological edge distribution: rebuild larger
        kw += 4
        in_maps = _prep(x, kernel, edge_vals, rows, cols, kw)
    if kw not in _cache:
        _cache[kw] = _build(kw)
    nc = _cache[kw]
    res = run_bass_kernel_spmd(nc, in_maps, core_ids=list(range(NCORES)))
    out = np.concatenate([res.results[c]["out"] for c in range(NCORES)], axis=0)
    return out.astype(np.float32)



# revision 2
# speedup vs baseline: 1.2036x; 1.2036x over previous
"""GCN layer (BN -> dense -> sparse softmax -> gather/scatter -> tanh) on 8
Trainium2 NeuronCores.

Strategy (two device programs, h-space aggregation, identity scatter):

 Program A (tiny): per core, load its 12500-node slab of x^T, compute BN
 partial sums, AllReduce them (the only collective), fold BN into the
 projection (W' = rstd*W, b' = -mean*rstd @ W), and emit
 h = BN(x) @ W  as fp16 [64, 12500] per core.

 Host (indexing only): edges are sharded by destination core. Within a
 core, destinations are sorted by degree and packed 128 per window, one
 PARTITION ROW per destination; window w holds K_w = max degree in the
 window chunks. The k-th edge of a destination sits at chunk k of its
 partition row, so the scatter matrix of every chunk is the IDENTITY.
 The host gathers h[col] for each edge slot into he_w [128, 65, K_w]
 (feature-major so the chunk axis is innermost; channel 64 is the edge
 mask that yields the softmax denominator).

 Program B: per window, ONE batched vector multiply by exp(val)
 (broadcast along the feature axis, innermost contiguous so the DVE
 packs 2 fp16/cycle), then K_w identity-stationary matmuls accumulate
 A[i, :] += exp*he into PSUM (the PE is used as a streaming adder; the
 stationary operand never changes). Flush: rec = 1/A[:,64] on vector,
 out = tanh(rec * A[:,0:64]) fused on scalar, DMA out fp16.

 Softmax needs no max subtraction (edge_vals are uniform [0,1)).
 Zero-degree destinations get one dummy mask=1,val=0 slot -> den=1,
 num=0 -> tanh(0)=0, matching segment_sum semantics.
"""
import sys

sys.path.insert(0, "/opt/trn_rl_repo")

import numpy as np
from contextlib import ExitStack

import concourse.bass as bass
import concourse.bacc as bacc
import concourse.mybir as mybir
import concourse.tile as tile
from concourse.bass_utils import run_bass_kernel_spmd

# problem constants
N = 100000
E = 1600000
F = 128
D = 64
BN_EPS = 1e-3
NCORES = 8
NPC = N // NCORES            # 12500 destination nodes per core
NW = (NPC + 127) // 128      # 98 windows per core (last window 84 dests)

f16, f32 = mybir.dt.float16, mybir.dt.float32

_cache_h: list = []          # [program] (fixed shape)
_cache_edge: dict = {}       # Ks tuple -> program


def _build_h():
    """Program A: BN stats + AllReduce + projection h^T = W'^T @ x^T + b'."""
    nc = bacc.Bacc(None, target_bir_lowering=False)

    xT = nc.declare_dram_parameter("xT", [F, NPC], f16, isOutput=False)
    w_in = nc.declare_dram_parameter("w_in", [F, D], f32, isOutput=False)
    hT = nc.declare_dram_parameter("hT", [D, NPC], f16, isOutput=True)

    with tile.TileContext(nc) as tc:
        with ExitStack() as ctx:
            sb = ctx.enter_context(tc.tile_pool(name="sb", bufs=1))
            pp = ctx.enter_context(tc.tile_pool(name="pp", bufs=1, space="PSUM"))
            dram = ctx.enter_context(tc.tile_pool(name="dram", bufs=1, space="DRAM"))

            xts = sb.tile([F, NPC], f16)
            nc.sync.dma_start(out=xts[:], in_=xT[:])

            stats = sb.tile([F, 2], f32)
            nc.vector.tensor_reduce(
                out=stats[:, 0:1], in_=xts[:], axis=mybir.AxisListType.X,
                op=mybir.AluOpType.add)
            sq_trash = sb.tile([F, NPC], f16)
            nc.scalar.activation(
                out=sq_trash[:], in_=xts[:],
                func=mybir.ActivationFunctionType.Square,
                accum_out=stats[:, 1:2])

            st_b = dram.tile([F, 2], f32)
            red_b = dram.tile([F, 2], f32)
            nc.gpsimd.dma_start(out=st_b[:], in_=stats[:])
            nc.gpsimd.collective_compute(
                "AllReduce", mybir.AluOpType.add,
                replica_groups=[list(range(NCORES))],
                ins=[st_b[:].opt()], outs=[red_b[:].opt()])
            red = sb.tile([F, 2], f32)
            nc.gpsimd.dma_start(out=red[:], in_=red_b[:])

            mean = sb.tile([F, 1], f32)
            nc.vector.tensor_scalar_mul(out=mean[:], in0=red[:, 0:1], scalar1=1.0 / N)
            ex2 = sb.tile([F, 1], f32)
            nc.vector.tensor_scalar_mul(out=ex2[:], in0=red[:, 1:2], scalar1=1.0 / N)
            msq = sb.tile([F, 1], f32)
            nc.vector.tensor_tensor(out=msq[:], in0=mean[:], in1=mean[:],
                                    op=mybir.AluOpType.mult)
            varep = sb.tile([F, 1], f32)
            nc.vector.tensor_tensor(out=varep[:], in0=ex2[:], in1=msq[:],
                                    op=mybir.AluOpType.subtract)
            nc.vector.tensor_scalar_add(out=varep[:], in0=varep[:], scalar1=BN_EPS)
            sdev = sb.tile([F, 1], f32)
            nc.scalar.activation(out=sdev[:], in_=varep[:],
                                 func=mybir.ActivationFunctionType.Sqrt)
            rstd = sb.tile([F, 1], f32)
            nc.vector.reciprocal(out=rstd[:], in_=sdev[:])

            w_sb = sb.tile([F, D], f32)
            nc.sync.dma_start(out=w_sb[:], in_=w_in[:])
            w16 = sb.tile([F, D], f16)
            nc.vector.tensor_scalar(out=w16[:], in0=w_sb[:], scalar1=rstd[:, 0:1],
                                    scalar2=None, op0=mybir.AluOpType.mult)
            nmr = sb.tile([F, 1], f32)
            nc.vector.tensor_tensor(out=nmr[:], in0=mean[:], in1=rstd[:],
                                    op=mybir.AluOpType.mult)
            nmr16 = sb.tile([F, 1], f16)
            nc.vector.tensor_scalar_mul(out=nmr16[:], in0=nmr[:], scalar1=-1.0)

            b_ps = pp.tile([D, 1], f32, tag="b")
            nc.tensor.matmul(out=b_ps[:], lhsT=w16[:], rhs=nmr16[:],
                             start=True, stop=True)
            bvec = sb.tile([D, 1], f32)
            nc.vector.tensor_copy(out=bvec[:], in_=b_ps[:])

            hT16 = sb.tile([D, NPC], f16)
            NT = 500
            for t in range(NPC // NT):
                s = t * NT
                hps = pp.tile([D, NT], f32, tag="h", bufs=4)
                nc.tensor.matmul(out=hps[:], lhsT=w16[:], rhs=xts[:, s:s + NT],
                                 start=True, stop=True)
                nc.scalar.activation(out=hT16[:, s:s + NT], in_=hps[:],
                                     func=mybir.ActivationFunctionType.Identity,
                                     bias=bvec[:, 0:1])
            nc.sync.dma_start(out=hT[:], in_=hT16[:])

    nc.finalize()
    return nc


def _build_edge(Ks):
    """Program B: per-window scaled accumulate + softmax-normalize + tanh."""
    Ks = list(Ks)
    offs = np.concatenate([[0], np.cumsum(Ks)]).astype(np.int64)
    TOT = int(offs[-1])

    nc = bacc.Bacc(None, target_bir_lowering=False)

    he_in = nc.declare_dram_parameter("he_in", [128, 65 * TOT], f16, isOutput=False)
    val_in = nc.declare_dram_parameter("val_in", [128, TOT], f16, isOutput=False)
    ident_in = nc.declare_dram_parameter("ident_in", [128, 128], f16, isOutput=False)
    out_p = nc.declare_dram_parameter("out", [NPC, D], f16, isOutput=True)

    with tile.TileContext(nc) as tc:
        with ExitStack() as ctx:
            sb = ctx.enter_context(tc.tile_pool(name="sb", bufs=1))
            pp = ctx.enter_context(tc.tile_pool(name="pp", bufs=1, space="PSUM"))

            ident_sb = sb.tile([128, 128], f16)
            nc.sync.dma_start(out=ident_sb[:], in_=ident_in[:])
            val_sb = sb.tile([128, TOT], f16)
            nc.sync.dma_start(out=val_sb[:], in_=val_in[:])
            exp_sb = sb.tile([128, TOT], f16)
            nc.scalar.activation(out=exp_sb[:], in_=val_sb[:],
                                 func=mybir.ActivationFunctionType.Exp)

            for w in range(NW):
                K = Ks[w]
                a = int(offs[w])
                m = min(128, NPC - w * 128)
                he_w = sb.tile([128, 65, K], f16, tag="he", bufs=3)
                nc.sync.dma_start(out=he_w[:],
                                  in_=he_in[:, 65 * a:65 * (a + K)])
                hp_w = sb.tile([128, 65, K], f16, tag="hp", bufs=3)
                nc.vector.tensor_tensor(
                    out=hp_w[:], in0=he_w[:],
                    in1=exp_sb[:, None, a:a + K].to_broadcast([128, 65, K]),
                    op=mybir.AluOpType.mult)
                A = pp.tile([128, 65], f32, tag="A", bufs=4)
                for k in range(K):
                    nc.tensor.matmul(out=A[:], lhsT=ident_sb[:],
                                     rhs=hp_w[:, :, k],
                                     start=(k == 0), stop=(k == K - 1))
                rec = sb.tile([128, 1], f32, tag="rec", bufs=4)
                nc.vector.reciprocal(out=rec[:], in_=A[:, 64:65])
                o16 = sb.tile([128, D], f16, tag="o", bufs=4)
                nc.scalar.activation(out=o16[:], in_=A[:, 0:D],
                                     func=mybir.ActivationFunctionType.Tanh,
                                     scale=rec[:, 0:1])
                nc.sync.dma_start(out=out_p[w * 128:w * 128 + m, :],
                                  in_=o16[:m, :])

    nc.finalize()
    return nc


def _edge_layout(rows, cols, edge_vals):
    """Degree-sorted identity layout. Host does indexing only."""
    order = np.argsort(rows, kind="stable")
    rs = rows[order].astype(np.int64)
    cs = cols[order].astype(np.int64)
    vs = edge_vals[order].astype(np.float16)

    core = rs // NPC
    loc = rs % NPC
    dest_global = core * NPC + loc

    deg = np.bincount(dest_global, minlength=N).reshape(NCORES, NPC)
    perm = np.argsort(-deg, axis=1, kind="stable")      # rank -> dest id
    rank_of = np.empty_like(perm)
    rows_idx = np.arange(NPC)
    for c in range(NCORES):
        rank_of[c, perm[c]] = rows_idx
    degs_sorted = -np.sort(-deg, axis=1)

    Kc = degs_sorted[:, ::128][:, :NW]                  # [NCORES, NW] window max deg
    Ks = Kc.max(axis=0)
    Ks = np.maximum(Ks, 2)
    Ks = ((Ks + 1) // 2) * 2                            # even (DVE 4B alignment)
    offs = np.concatenate([[0], np.cumsum(Ks)]).astype(np.int64)
    TOT = int(offs[-1])

    counts = np.bincount(dest_global, minlength=N)
    starts = np.zeros(N, np.int64)
    np.cumsum(counts[:-1], out=starts[1:])
    k_idx = np.arange(len(rs)) - starts[dest_global]

    r = rank_of[core, loc]
    wi = r // 128
    pi = r % 128
    slot = offs[wi] + k_idx

    colf = np.full((NCORES, 128, TOT), N, np.int64)     # N -> zero row
    valf = np.zeros((NCORES, 128, TOT), np.float16)
    mask = np.zeros((NCORES, 128, TOT), np.float16)
    colf[core, pi, slot] = cs
    valf[core, pi, slot] = vs
    mask[core, pi, slot] = 1.0

    # zero-degree dests: one dummy slot with mask=1, val=0 -> den=1, num=0
    for c in range(NCORES):
        zr = np.nonzero(degs_sorted[c] == 0)[0]
        if len(zr):
            mask[c, zr % 128, offs[zr // 128]] = 1.0

    return perm, Ks, offs, TOT, colf, valf, mask


def _build_he(h16ext, colf_c, mask_c, Ks, offs, TOT):
    """he_in for one core: per window [128, 65, K] blocks, flattened."""
    g = h16ext[colf_c]                                  # [128, TOT, 64]
    he = np.empty((128, 65 * TOT), np.float16)
    for w in range(NW):
        a, b = int(offs[w]), int(offs[w + 1])
        blk = np.empty((128, 65, b - a), np.float16)
        blk[:, 0:D, :] = np.swapaxes(g[:, a:b, :], 1, 2)
        blk[:, D, :] = mask_c[:, a:b]
        he[:, 65 * a:65 * b] = blk.reshape(128, -1)
    return he


def kernel(x, kernel, edge_vals, rows, cols, nodes_num):
    assert int(nodes_num) == N and x.shape == (N, F) and kernel.shape == (F, D)
    x = np.asarray(x, dtype=np.float32)
    kernel = np.ascontiguousarray(np.asarray(kernel, dtype=np.float32))
    edge_vals = np.asarray(edge_vals, dtype=np.float32)
    rows = np.asarray(rows)
    cols = np.asarray(cols)

    # ---- program A: h = BN(x) @ W ----
    if not _cache_h:
        _cache_h.append(_build_h())
    nc_h = _cache_h[0]
    x16 = x.astype(np.float16)
    in_maps_h = [{
        "xT": np.ascontiguousarray(x16[c * NPC:(c + 1) * NPC, :].T),
        "w_in": kernel,
    } for c in range(NCORES)]
    res_h = run_bass_kernel_spmd(nc_h, in_maps_h, core_ids=list(range(NCORES)))
    h16 = np.concatenate(
        [res_h.results[c]["hT"].T for c in range(NCORES)], axis=0)  # [N, 64]

    # ---- host: edge layout + gather (indexing only) ----
    perm, Ks, offs, TOT, colf, valf, mask = _edge_layout(rows, cols, edge_vals)
    key = tuple(int(k) for k in Ks)
    if key not in _cache_edge:
        _cache_edge[key] = _build_edge(Ks)
    nc_e = _cache_edge[key]

    h16ext = np.vstack([h16, np.zeros((1, D), np.float16)])
    ident = np.eye(128, dtype=np.float16)
    in_maps_e = []
    for c in range(NCORES):
        in_maps_e.append({
            "he_in": _build_he(h16ext, colf[c], mask[c], Ks, offs, TOT),
            "val_in": np.ascontiguousarray(valf[c]),
            "ident_in": ident,
        })
    res_e = run_bass_kernel_spmd(nc_e, in_maps_e, core_ids=list(range(NCORES)))

    out = np.empty((N, D), np.float32)
    for c in range(NCORES):
        o = res_e.results[c]["out"].astype(np.float32)  # [NPC, 64] rank order
        out[c * NPC + perm[c], :] = o
    return out


# revision 11
# speedup vs baseline: 1.8660x; 1.5504x over previous
"""GCN layer (BN -> dense -> sparse softmax -> gather/scatter -> tanh) on 8
Trainium2 NeuronCores.

Strategy (two device programs, h-space aggregation, identity scatter):

 Program A (tiny): per core, load its 12500-node slab of x^T, compute BN
 partial sums, AllReduce them (the only collective), fold BN into the
 projection (W' = rstd*W, b' = -mean*rstd @ W), and emit
 h = BN(x) @ W  as fp16 [64, 12500] per core.

 Host (indexing only): edges are sharded by destination core. Within a
 core, destinations are sorted by degree and packed 128 per window, one
 PARTITION ROW per destination; window w holds K_w = max degree in the
 window chunks. The k-th edge of a destination sits at chunk k of its
 partition row, so the scatter matrix of every chunk is the IDENTITY.
 The host gathers h[col] for each edge slot into he_w [128, 65, K_w]
 (feature-major so the chunk axis is innermost; channel 64 is the edge
 mask that yields the softmax denominator).

 Program B: per window, ONE batched vector multiply by exp(val)
 (broadcast along the feature axis, innermost contiguous so the DVE
 packs 2 fp16/cycle), then K_w identity-stationary matmuls accumulate
 A[i, :] += exp*he into PSUM (the PE is used as a streaming adder; the
 stationary operand never changes). Flush: rec = 1/A[:,64] on vector,
 out = tanh(rec * A[:,0:64]) fused on scalar, DMA out fp16.

 Softmax needs no max subtraction (edge_vals are uniform [0,1)).
 Zero-degree destinations get one dummy mask=1,val=0 slot -> den=1,
 num=0 -> tanh(0)=0, matching segment_sum semantics.
"""
import sys

sys.path.insert(0, "/opt/trn_rl_repo")

import numpy as np
from contextlib import ExitStack

import concourse.bass as bass
import concourse.bacc as bacc
import concourse.mybir as mybir
import concourse.tile as tile
from concourse.bass_utils import run_bass_kernel_spmd

# problem constants
N = 100000
E = 1600000
F = 128
D = 64
BN_EPS = 1e-3
NCORES = 8
NPC = N // NCORES            # 12500 destination nodes per core
NW = (NPC + 127) // 128      # 98 windows per core (last window 84 dests)

f16, f32 = mybir.dt.float16, mybir.dt.float32

_cache_h: list = []          # [program] (fixed shape)
_cache_edge: dict = {}       # Ks tuple -> program


def _round_up_size(size):
    for valid in (32, 64, 128):
        if valid >= size:
            return valid
    raise AssertionError(size)


def _mm(nc, out, lhsT, rhs, start, stop, ldw):
    """matmul with explicit control of the ldweights flag (stationary reuse)."""
    te = nc.tensor
    ifmap_ap = te.lower_ap(rhs.opt({0}), opt=False)
    weights_ap = te.lower_ap(lhsT.opt({0}), opt=False, for_matmul_weights=True)
    out_ap = te.lower_ap(out)
    tile_size = (_round_up_size(rhs.partition_size()),
                 _round_up_size(out.partition_size()))
    inst = mybir.InstMatmult(
        name=te.bass.get_next_instruction_name(),
        replication_resolution=0,
        replication_shift_amnt=0,
        replication_num_rows=0,
        start_tensor_calc=start,
        stop_tensor_calc=stop,
        ins=[ifmap_ap, weights_ap],
        outs=[out_ap],
        perf_mode=None,
        is_transpose=None,
        ifmap_quant_offset=None,
        weights_quant_offset=None,
        bass_skip_group_check=False,
        tile_position=(0, 0),
        tile_size=tile_size,
        ldweights=ldw,
    )
    return te.add_instruction(inst)


def _build_h():
    """Program A: BN stats + AllReduce + projection h^T = W'^T @ x^T + b'."""
    nc = bacc.Bacc(None, target_bir_lowering=False)

    xT = nc.declare_dram_parameter("xT", [F, NPC], f16, isOutput=False)
    w_in = nc.declare_dram_parameter("w_in", [F, D], f32, isOutput=False)
    hT = nc.declare_dram_parameter("hT", [D, NPC], f16, isOutput=True)

    with tile.TileContext(nc) as tc:
        with ExitStack() as ctx:
            sb = ctx.enter_context(tc.tile_pool(name="sb", bufs=1))
            pp = ctx.enter_context(tc.tile_pool(name="pp", bufs=1, space="PSUM"))
            dram = ctx.enter_context(tc.tile_pool(name="dram", bufs=1, space="DRAM"))

            xts = sb.tile([F, NPC], f16)
            half = NPC // 2
            nc.sync.dma_start(out=xts[:, 0:half], in_=xT[:, 0:half])
            nc.gpsimd.dma_start(out=xts[:, half:], in_=xT[:, half:])

            stats = sb.tile([F, 2], f32)
            nc.vector.tensor_reduce(
                out=stats[:, 0:1], in_=xts[:], axis=mybir.AxisListType.X,
                op=mybir.AluOpType.add)
            sq_trash = sb.tile([F, NPC], f16)
            nc.scalar.activation(
                out=sq_trash[:], in_=xts[:],
                func=mybir.ActivationFunctionType.Square,
                accum_out=stats[:, 1:2])

            st_b = dram.tile([F, 2], f32)
            red_b = dram.tile([F, 2], f32)
            nc.gpsimd.dma_start(out=st_b[:], in_=stats[:])
            nc.gpsimd.collective_compute(
                "AllReduce", mybir.AluOpType.add,
                replica_groups=[list(range(NCORES))],
                ins=[st_b[:].opt()], outs=[red_b[:].opt()])
            red = sb.tile([F, 2], f32)
            nc.gpsimd.dma_start(out=red[:], in_=red_b[:])

            mean = sb.tile([F, 1], f32)
            nc.vector.tensor_scalar_mul(out=mean[:], in0=red[:, 0:1], scalar1=1.0 / N)
            ex2 = sb.tile([F, 1], f32)
            nc.vector.tensor_scalar_mul(out=ex2[:], in0=red[:, 1:2], scalar1=1.0 / N)
            msq = sb.tile([F, 1], f32)
            nc.vector.tensor_tensor(out=msq[:], in0=mean[:], in1=mean[:],
                                    op=mybir.AluOpType.mult)
            varep = sb.tile([F, 1], f32)
            nc.vector.tensor_tensor(out=varep[:], in0=ex2[:], in1=msq[:],
                                    op=mybir.AluOpType.subtract)
            nc.vector.tensor_scalar_add(out=varep[:], in0=varep[:], scalar1=BN_EPS)
            sdev = sb.tile([F, 1], f32)
            nc.scalar.activation(out=sdev[:], in_=varep[:],
                                 func=mybir.ActivationFunctionType.Sqrt)
            rstd = sb.tile([F, 1], f32)
            nc.vector.reciprocal(out=rstd[:], in_=sdev[:])

            w_sb = sb.tile([F, D], f32)
            nc.sync.dma_start(out=w_sb[:], in_=w_in[:])
            w16 = sb.tile([F, D], f16)
            nc.vector.tensor_scalar(out=w16[:], in0=w_sb[:], scalar1=rstd[:, 0:1],
                                    scalar2=None, op0=mybir.AluOpType.mult)
            nmr = sb.tile([F, 1], f32)
            nc.vector.tensor_tensor(out=nmr[:], in0=mean[:], in1=rstd[:],
                                    op=mybir.AluOpType.mult)
            nmr16 = sb.tile([F, 1], f16)
            nc.vector.tensor_scalar_mul(out=nmr16[:], in0=nmr[:], scalar1=-1.0)

            b_ps = pp.tile([D, 1], f32, tag="b")
            nc.tensor.matmul(out=b_ps[:], lhsT=w16[:], rhs=nmr16[:],
                             start=True, stop=True)
            bvec = sb.tile([D, 1], f32)
            nc.vector.tensor_copy(out=bvec[:], in_=b_ps[:])

            hRaw = sb.tile([D, NPC], f32)
            hT16 = sb.tile([D, NPC], f16)
            NT = 500
            NCHUNK = NPC // NT
            nc.tensor.ldweights(w16[:])
            for t in range(NCHUNK):
                s = t * NT
                hps = pp.tile([D, NT], f32, tag="h", bufs=4)
                _mm(nc, out=hps[:], lhsT=w16[:], rhs=xts[:, s:s + NT],
                    start=True, stop=True, ldw=False)
                nc.vector.tensor_copy(out=hRaw[:, s:s + NT], in_=hps[:])
            # one fused bias-add + fp16 cast over the whole slab, then 2 DMAs
            nc.scalar.activation(out=hT16[:, 0:half], in_=hRaw[:, 0:half],
                                 func=mybir.ActivationFunctionType.Identity,
                                 bias=bvec[:, 0:1])
            nc.scalar.activation(out=hT16[:, half:], in_=hRaw[:, half:],
                                 func=mybir.ActivationFunctionType.Identity,
                                 bias=bvec[:, 0:1])
            nc.sync.dma_start(out=hT[:, 0:half], in_=hT16[:, 0:half])
            nc.gpsimd.dma_start(out=hT[:, half:], in_=hT16[:, half:])

    nc.finalize()
    return nc


def _build_edge(Ks):
    """Program B: per-window scaled accumulate + softmax-normalize + tanh."""
    Ks = list(Ks)
    offs = np.concatenate([[0], np.cumsum(Ks)]).astype(np.int64)
    TOT = int(offs[-1])

    nc = bacc.Bacc(None, target_bir_lowering=False)

    he_in = nc.declare_dram_parameter("he_in", [128, 65 * TOT], f16, isOutput=False)
    val_in = nc.declare_dram_parameter("val_in", [128, TOT], f16, isOutput=False)
    ident_in = nc.declare_dram_parameter("ident_in", [128, 128], f16, isOutput=False)
    out_p = nc.declare_dram_parameter("out", [NPC, D], f16, isOutput=True)

    queues = [nc.sync, nc.gpsimd]

    with tile.TileContext(nc) as tc:
        with ExitStack() as ctx:
            sb = ctx.enter_context(tc.tile_pool(name="sb", bufs=1))
            pp = ctx.enter_context(tc.tile_pool(name="pp", bufs=1, space="PSUM"))

            ident_sb = sb.tile([128, 128], f16)
            nc.sync.dma_start(out=ident_sb[:], in_=ident_in[:])
            val_sb = sb.tile([128, TOT], f16)
            nc.sync.dma_start(out=val_sb[:], in_=val_in[:])
            exp_sb = sb.tile([128, TOT], f16)
            nc.scalar.activation(out=exp_sb[:], in_=val_sb[:],
                                 func=mybir.ActivationFunctionType.Exp)

            nc.tensor.ldweights(ident_sb[:])
            for w in range(NW):
                K = Ks[w]
                a = int(offs[w])
                m = min(128, NPC - w * 128)
                q = queues[w % 2]
                qo = queues[(w + 1) % 2]
                he_w = sb.tile([128, 65, K], f16, tag="he", bufs=4)
                q.dma_start(out=he_w[:], in_=he_in[:, 65 * a:65 * (a + K)])
                hp_w = sb.tile([128, 65, K], f16, tag="hp", bufs=4)
                nc.vector.tensor_tensor(
                    out=hp_w[:], in0=he_w[:],
                    in1=exp_sb[:, None, a:a + K].to_broadcast([128, 65, K]),
                    op=mybir.AluOpType.mult)
                A = pp.tile([128, 65, 2], f32, tag="A", bufs=4)
                nmm = K // 2
                for j in range(nmm):
                    _mm(nc, out=A[:], lhsT=ident_sb[:],
                        rhs=hp_w[:, :, 2 * j:2 * j + 2],
                        start=(j == 0), stop=(j == nmm - 1),
                        ldw=False)
                Af = sb.tile([128, 65], f32, tag="Af", bufs=4)
                nc.vector.tensor_reduce(out=Af[:, :, None], in_=A[:],
                                        axis=mybir.AxisListType.X,
                                        op=mybir.AluOpType.add)
                rec = sb.tile([128, 1], f32, tag="rec", bufs=4)
                nc.vector.reciprocal(out=rec[:], in_=Af[:, 64:65])
                o16 = sb.tile([128, D], f16, tag="o", bufs=4)
                nc.scalar.activation(out=o16[:], in_=Af[:, 0:D],
                                     func=mybir.ActivationFunctionType.Tanh,
                                     scale=rec[:, 0:1])
                qo.dma_start(out=out_p[w * 128:w * 128 + m, :],
                             in_=o16[:m, :])

    nc.finalize()
    return nc


def _edge_layout(rows, cols, edge_vals):
    """Degree-sorted identity layout. Host does indexing only."""
    order = np.argsort(rows, kind="stable")
    rs = rows[order].astype(np.int64)
    cs = cols[order].astype(np.int64)
    vs = edge_vals[order].astype(np.float16)

    core = rs // NPC
    loc = rs % NPC
    dest_global = core * NPC + loc

    deg = np.bincount(dest_global, minlength=N).reshape(NCORES, NPC)
    perm = np.argsort(-deg, axis=1, kind="stable")      # rank -> dest id
    rank_of = np.empty_like(perm)
    rows_idx = np.arange(NPC)
    for c in range(NCORES):
        rank_of[c, perm[c]] = rows_idx
    degs_sorted = -np.sort(-deg, axis=1)

    Kc = degs_sorted[:, ::128][:, :NW]                  # [NCORES, NW] window max deg
    Ks = Kc.max(axis=0)
    Ks = np.maximum(Ks, 2)
    Ks = ((Ks + 1) // 2) * 2                            # even (DVE 4B alignment)
    offs = np.concatenate([[0], np.cumsum(Ks)]).astype(np.int64)
    TOT = int(offs[-1])

    counts = np.bincount(dest_global, minlength=N)
    starts = np.zeros(N, np.int64)
    np.cumsum(counts[:-1], out=starts[1:])
    k_idx = np.arange(len(rs)) - starts[dest_global]

    r = rank_of[core, loc]
    wi = r // 128
    pi = r % 128
    slot = offs[wi] + k_idx

    colf = np.full((NCORES, 128, TOT), N, np.int64)     # N -> zero row
    valf = np.zeros((NCORES, 128, TOT), np.float16)
    mask = np.zeros((NCORES, 128, TOT), np.float16)
    colf[core, pi, slot] = cs
    valf[core, pi, slot] = vs
    mask[core, pi, slot] = 1.0

    # zero-degree dests: one dummy slot with mask=1, val=0 -> den=1, num=0
    for c in range(NCORES):
        zr = np.nonzero(degs_sorted[c] == 0)[0]
        if len(zr):
            mask[c, zr % 128, offs[zr // 128]] = 1.0

    return perm, Ks, offs, TOT, colf, valf, mask


def _build_he(h16ext, colf_c, mask_c, Ks, offs, TOT):
    """he_in for one core: per window [128, 65, K] blocks, flattened."""
    g = h16ext[colf_c]                                  # [128, TOT, 64]
    he = np.empty((128, 65 * TOT), np.float16)
    for w in range(NW):
        a, b = int(offs[w]), int(offs[w + 1])
        blk = np.empty((128, 65, b - a), np.float16)
        blk[:, 0:D, :] = np.swapaxes(g[:, a:b, :], 1, 2)
        blk[:, D, :] = mask_c[:, a:b]
        he[:, 65 * a:65 * b] = blk.reshape(128, -1)
    return he


def kernel(x, kernel, edge_vals, rows, cols, nodes_num):
    assert int(nodes_num) == N and x.shape == (N, F) and kernel.shape == (F, D)
    x = np.asarray(x, dtype=np.float32)
    kernel = np.ascontiguousarray(np.asarray(kernel, dtype=np.float32))
    edge_vals = np.asarray(edge_vals, dtype=np.float32)
    rows = np.asarray(rows)
    cols = np.asarray(cols)

    # ---- program A: h = BN(x) @ W ----
    if not _cache_h:
        _cache_h.append(_build_h())
    nc_h = _cache_h[0]
    x16 = x.astype(np.float16)
    in_maps_h = [{
        "xT": np.ascontiguousarray(x16[c * NPC:(c + 1) * NPC, :].T),
        "w_in": kernel,
    } for c in range(NCORES)]
    res_h = run_bass_kernel_spmd(nc_h, in_maps_h, core_ids=list(range(NCORES)))
    h16 = np.concatenate(
        [res_h.results[c]["hT"].T for c in range(NCORES)], axis=0)  # [N, 64]

    # ---- host: edge layout + gather (indexing only) ----
    perm, Ks, offs, TOT, colf, valf, mask = _edge_layout(rows, cols, edge_vals)
    key = tuple(int(k) for k in Ks)
    if key not in _cache_edge:
        _cache_edge[key] = _build_edge(Ks)
    nc_e = _cache_edge[key]

    h16ext = np.vstack([h16, np.zeros((1, D), np.float16)])
    ident = np.eye(128, dtype=np.float16)
    in_maps_e = []
    for c in range(NCORES):
        in_maps_e.append({
            "he_in": _build_he(h16ext, colf[c], mask[c], Ks, offs, TOT),
            "val_in": np.ascontiguousarray(valf[c]),
            "ident_in": ident,
        })
    res_e = run_bass_kernel_spmd(nc_e, in_maps_e, core_ids=list(range(NCORES)))

    out = np.empty((N, D), np.float32)
    for c in range(NCORES):
        o = res_e.results[c]["out"].astype(np.float32)  # [NPC, 64] rank order
        out[c * NPC + perm[c], :] = o
    return out


# revision 12
# speedup vs baseline: 2.3517x; 1.2603x over previous
"""GCN layer (BN -> dense -> sparse softmax -> gather/scatter -> tanh) on 8
Trainium2 NeuronCores.

Strategy (three small device programs, h-space aggregation, identity scatter):

 Program A1: per core, load its 12500-node slab of x^T and emit BN partial
 sums [128, 2] (sum, sum of squares). No collective — the cross-core
 reduction rides the host round-trip that the edge gather needs anyway
 (host only CONCATENATES the 8 partial tiles; all arithmetic on device).

 Program A2: per core, read all 8 partial-stat tiles, finish mean/rstd,
 fold BN into the projection (W' = rstd*W, b' = -mean*rstd @ W), and emit
 h = BN(x) @ W as fp16 [64, 12500] for its slab.

 Host (indexing only): edges are sharded by destination core. Within a
 core, destinations are sorted by degree and packed 128 per window, one
 PARTITION ROW per destination; window w holds K_w = max degree in the
 window chunks. The k-th edge of a destination sits at chunk k of its
 partition row, so the scatter matrix of every chunk is the IDENTITY.
 The host gathers h[col] for each edge slot into he_w [128, 65, K_w]
 (feature-major, chunk axis innermost; channel 64 is the edge mask that
 yields the softmax denominator).

 Program B: windows are processed in groups of 4 (one ~1 MB input DMA and
 one output DMA per group, alternating across two queues). Per window:
 ONE batched vector multiply by exp(val) (broadcast along the feature
 axis; innermost contiguous so the DVE packs 2 fp16/cycle), then K/4
 identity-stationary matmuls accumulate 4 chunks each into PSUM
 [128, 65, 4]; a vector reduce folds the 4 lanes; reciprocal is batched
 per group; out = tanh(rec * A[:, 0:64]) fused on the scalar engine.

 Softmax needs no max subtraction (edge_vals are uniform [0,1)).
 Zero-degree destinations get one dummy mask=1,val=0 slot -> den=1,
 num=0 -> tanh(0)=0, matching segment_sum semantics.
"""
import sys

sys.path.insert(0, "/opt/trn_rl_repo")

import numpy as np
from contextlib import ExitStack

import concourse.bass as bass
import concourse.bacc as bacc
import concourse.mybir as mybir
import concourse.tile as tile
from concourse.bass_utils import run_bass_kernel_spmd

# problem constants
N = 100000
E = 1600000
F = 128
D = 64
BN_EPS = 1e-3
NCORES = 8
NPC = N // NCORES            # 12500 destination nodes per core
NW = (NPC + 127) // 128      # 98 windows per core (last window 84 dests)
GW = 4                       # windows per DMA group
GMM = 4                      # chunks folded per matmul (PSUM [128, 65, GMM])

f16, f32 = mybir.dt.float16, mybir.dt.float32

_cache: dict = {}            # name -> program


def _round_up_size(size):
    for valid in (32, 64, 128):
        if valid >= size:
            return valid
    raise AssertionError(size)


def _mm(nc, out, lhsT, rhs, start, stop, ldw):
    """matmul with explicit control of the ldweights flag."""
    te = nc.tensor
    ifmap_ap = te.lower_ap(rhs.opt({0}), opt=False)
    weights_ap = te.lower_ap(lhsT.opt({0}), opt=False, for_matmul_weights=True)
    out_ap = te.lower_ap(out)
    tile_size = (_round_up_size(rhs.partition_size()),
                 _round_up_size(out.partition_size()))
    inst = mybir.InstMatmult(
        name=te.bass.get_next_instruction_name(),
        replication_resolution=0,
        replication_shift_amnt=0,
        replication_num_rows=0,
        start_tensor_calc=start,
        stop_tensor_calc=stop,
        ins=[ifmap_ap, weights_ap],
        outs=[out_ap],
        perf_mode=None,
        is_transpose=None,
        ifmap_quant_offset=None,
        weights_quant_offset=None,
        bass_skip_group_check=False,
        tile_position=(0, 0),
        tile_size=tile_size,
        ldweights=ldw,
    )
    return te.add_instruction(inst)


def _build_stats():
    """Program A1: per-core BN partial sums [128, 2] = [sum, sumsq]."""
    nc = bacc.Bacc(None, target_bir_lowering=False)
    xT = nc.declare_dram_parameter("xT", [F, NPC], f16, isOutput=False)
    st_out = nc.declare_dram_parameter("st", [F, 2], f32, isOutput=True)

    with tile.TileContext(nc) as tc:
        with ExitStack() as ctx:
            sb = ctx.enter_context(tc.tile_pool(name="sb", bufs=1))

            xts = sb.tile([F, NPC], f16)
            half = NPC // 2
            nc.sync.dma_start(out=xts[:, 0:half], in_=xT[:, 0:half])
            nc.gpsimd.dma_start(out=xts[:, half:], in_=xT[:, half:])

            stats = sb.tile([F, 2], f32)
            # sums: first half on vector, second half via scalar accum pass
            s1 = sb.tile([F, 1], f32)
            nc.vector.tensor_reduce(
                out=s1[:], in_=xts[:, 0:half], axis=mybir.AxisListType.X,
                op=mybir.AluOpType.add)
            s2 = sb.tile([F, 1], f32)
            cp_trash = sb.tile([F, NPC - half], f16)
            nc.scalar.activation(
                out=cp_trash[:], in_=xts[:, half:],
                func=mybir.ActivationFunctionType.Copy,
                accum_out=s2[:])
            nc.vector.tensor_tensor(out=stats[:, 0:1], in0=s1[:], in1=s2[:],
                                    op=mybir.AluOpType.add)
            sq_trash = sb.tile([F, NPC], f16)
            nc.scalar.activation(
                out=sq_trash[:], in_=xts[:],
                func=mybir.ActivationFunctionType.Square,
                accum_out=stats[:, 1:2])
            nc.sync.dma_start(out=st_out[:], in_=stats[:])

    nc.finalize()
    return nc


def _build_h():
    """Program A2: finish BN from all-core partials, project h = BN(x)@W."""
    nc = bacc.Bacc(None, target_bir_lowering=False)
    xT = nc.declare_dram_parameter("xT", [F, NPC], f16, isOutput=False)
    w_in = nc.declare_dram_parameter("w_in", [F, D], f32, isOutput=False)
    stats_in = nc.declare_dram_parameter("stats_in", [F, NCORES * 2], f32,
                                         isOutput=False)
    hT = nc.declare_dram_parameter("hT", [D, NPC], f16, isOutput=True)

    NT = 500
    NCHUNK = NPC // NT

    with tile.TileContext(nc) as tc:
        with ExitStack() as ctx:
            sb = ctx.enter_context(tc.tile_pool(name="sb", bufs=1))
            pp = ctx.enter_context(tc.tile_pool(name="pp", bufs=1, space="PSUM"))

            stp = sb.tile([F, NCORES, 2], f32)
            nc.sync.dma_start(out=stp[:], in_=stats_in[:])
            red = sb.tile([F, 2], f32)
            # sum over the 8 cores: view [F, 2, 8] (stride trick) reduce X
            nc.vector.tensor_reduce(
                out=red[:, :, None],
                in_=stp[:].rearrange("p c s -> p s c"),
                axis=mybir.AxisListType.X, op=mybir.AluOpType.add)

            w_sb = sb.tile([F, D], f32)
            nc.gpsimd.dma_start(out=w_sb[:], in_=w_in[:])
            xts = sb.tile([F, NPC], f16)
            half = NPC // 2
            nc.sync.dma_start(out=xts[:, 0:half], in_=xT[:, 0:half])
            nc.gpsimd.dma_start(out=xts[:, half:], in_=xT[:, half:])

            mean = sb.tile([F, 1], f32)
            nc.vector.tensor_scalar_mul(out=mean[:], in0=red[:, 0:1],
                                        scalar1=1.0 / N)
            ex2 = sb.tile([F, 1], f32)
            nc.vector.tensor_scalar_mul(out=ex2[:], in0=red[:, 1:2],
                                        scalar1=1.0 / N)
            msq = sb.tile([F, 1], f32)
            nc.vector.tensor_tensor(out=msq[:], in0=mean[:], in1=mean[:],
                                    op=mybir.AluOpType.mult)
            varep = sb.tile([F, 1], f32)
            nc.vector.tensor_tensor(out=varep[:], in0=ex2[:], in1=msq[:],
                                    op=mybir.AluOpType.subtract)
            nc.vector.tensor_scalar_add(out=varep[:], in0=varep[:],
                                        scalar1=BN_EPS)
            sdev = sb.tile([F, 1], f32)
            nc.scalar.activation(out=sdev[:], in_=varep[:],
                                 func=mybir.ActivationFunctionType.Sqrt)
            rstd = sb.tile([F, 1], f32)
            nc.vector.reciprocal(out=rstd[:], in_=sdev[:])

            w16 = sb.tile([F, D], f16)
            nc.vector.tensor_scalar(out=w16[:], in0=w_sb[:],
                                    scalar1=rstd[:, 0:1], scalar2=None,
                                    op0=mybir.AluOpType.mult)
            nmr = sb.tile([F, 1], f32)
            nc.vector.tensor_tensor(out=nmr[:], in0=mean[:], in1=rstd[:],
                                    op=mybir.AluOpType.mult)
            nmr16 = sb.tile([F, 1], f16)
            nc.vector.tensor_scalar_mul(out=nmr16[:], in0=nmr[:], scalar1=-1.0)

            b_ps = pp.tile([D, 1], f32, tag="b")
            nc.tensor.matmul(out=b_ps[:], lhsT=w16[:], rhs=nmr16[:],
                             start=True, stop=True)
            bvec = sb.tile([D, 1], f32)
            nc.vector.tensor_copy(out=bvec[:], in_=b_ps[:])

            hT16 = sb.tile([D, NPC], f16)
            nc.tensor.ldweights(w16[:])
            for t in range(NCHUNK):
                s = t * NT
                hps = pp.tile([D, NT], f32, tag="h", bufs=4)
                _mm(nc, out=hps[:], lhsT=w16[:], rhs=xts[:, s:s + NT],
                    start=True, stop=True, ldw=False)
                nc.scalar.activation(out=hT16[:, s:s + NT], in_=hps[:],
                                     func=mybir.ActivationFunctionType.Identity,
                                     bias=bvec[:, 0:1])
                if t % 5 == 4:
                    q = nc.sync if (t // 5) % 2 == 0 else nc.gpsimd
                    q.dma_start(out=hT[:, s - 4 * NT:s + NT],
                                in_=hT16[:, s - 4 * NT:s + NT])

    nc.finalize()
    return nc


def _build_edge(Ks):
    """Program B: per-window scaled accumulate + softmax-normalize + tanh."""
    Ks = list(Ks)
    offs = np.concatenate([[0], np.cumsum(Ks)]).astype(np.int64)
    TOT = int(offs[-1])

    nc = bacc.Bacc(None, target_bir_lowering=False)

    he_in = nc.declare_dram_parameter("he_in", [128, 65 * TOT], f16,
                                      isOutput=False)
    val_in = nc.declare_dram_parameter("val_in", [128, TOT], f16,
                                       isOutput=False)
    ident_in = nc.declare_dram_parameter("ident_in", [128, 128], f16,
                                         isOutput=False)
    out_p = nc.declare_dram_parameter("out", [NW * 128, D], f16, isOutput=True)

    queues = [nc.sync, nc.gpsimd]
    groups = [list(range(g0, min(g0 + GW, NW))) for g0 in range(0, NW, GW)]

    with tile.TileContext(nc) as tc:
        with ExitStack() as ctx:
            sb = ctx.enter_context(tc.tile_pool(name="sb", bufs=1))
            pp = ctx.enter_context(tc.tile_pool(name="pp", bufs=1, space="PSUM"))

            ident_sb = sb.tile([128, 128], f16)
            nc.sync.dma_start(out=ident_sb[:], in_=ident_in[:])
            val_sb = sb.tile([128, TOT], f16)
            nc.sync.dma_start(out=val_sb[:], in_=val_in[:])
            exp_sb = sb.tile([128, TOT], f16)
            nc.scalar.activation(out=exp_sb[:], in_=val_sb[:],
                                 func=mybir.ActivationFunctionType.Exp)

            nc.tensor.ldweights(ident_sb[:])
            for gi, gwin in enumerate(groups):
                q = queues[gi % 2]
                qo = queues[(gi + 1) % 2]
                a = int(offs[gwin[0]])
                b = int(offs[gwin[-1] + 1])
                GK = b - a
                ng = len(gwin)
                he_g = sb.tile([128, 65 * GK], f16, tag="he", bufs=3)
                q.dma_start(out=he_g[:], in_=he_in[:, 65 * a:65 * b])
                og = sb.tile([128, ng, D], f16, tag="og", bufs=3)
                Afg = sb.tile([128, ng, 65], f32, tag="Af", bufs=3)
                for wi, w in enumerate(gwin):
                    K = Ks[w]
                    ca = int(offs[w]) - a
                    he_w = he_g[:, 65 * ca:65 * (ca + K)].rearrange(
                        "p (f k) -> p f k", k=K)
                    hp_w = sb.tile([128, 65, K], f16, tag="hp", bufs=4)
                    nc.vector.tensor_tensor(
                        out=hp_w[:], in0=he_w,
                        in1=exp_sb[:, None, a + ca:a + ca + K]
                            .to_broadcast([128, 65, K]),
                        op=mybir.AluOpType.mult)
                    A = pp.tile([128, 65, GMM], f32, tag="A", bufs=4)
                    nmm = K // GMM
                    for j in range(nmm):
                        _mm(nc, out=A[:], lhsT=ident_sb[:],
                            rhs=hp_w[:, :, GMM * j:GMM * (j + 1)],
                            start=(j == 0), stop=(j == nmm - 1),
                            ldw=False)
                    nc.vector.tensor_reduce(
                        out=Afg[:, wi, :, None], in_=A[:],
                        axis=mybir.AxisListType.X, op=mybir.AluOpType.add)
                rec_g = sb.tile([128, ng], f32, tag="rec", bufs=3)
                nc.vector.reciprocal(out=rec_g[:],
                                     in_=Afg[:, :, 64])
                for wi, w in enumerate(gwin):
                    nc.scalar.activation(out=og[:, wi, :],
                                         in_=Afg[:, wi, 0:D],
                                         func=mybir.ActivationFunctionType.Tanh,
                                         scale=rec_g[:, wi:wi + 1])
                g0 = gwin[0]
                qo.dma_start(
                    out=out_p[g0 * 128:(g0 + ng) * 128, :]
                        .rearrange("(w p) f -> p w f", w=ng),
                    in_=og[:])

    nc.finalize()
    return nc


def _edge_layout(rows, cols, edge_vals):
    """Degree-sorted identity layout. Host does indexing only."""
    order = np.argsort(rows, kind="stable")
    rs = rows[order].astype(np.int64)
    cs = cols[order].astype(np.int64)
    vs = edge_vals[order].astype(np.float16)

    core = rs // NPC
    loc = rs % NPC
    dest_global = core * NPC + loc

    deg = np.bincount(dest_global, minlength=N).reshape(NCORES, NPC)
    perm = np.argsort(-deg, axis=1, kind="stable")      # rank -> dest id
    rank_of = np.empty_like(perm)
    rows_idx = np.arange(NPC)
    for c in range(NCORES):
        rank_of[c, perm[c]] = rows_idx
    degs_sorted = -np.sort(-deg, axis=1)

    Kc = degs_sorted[:, ::128][:, :NW]                  # [NCORES, NW]
    Ks = Kc.max(axis=0)
    Ks = np.maximum(Ks, GMM)
    Ks = ((Ks + GMM - 1) // GMM) * GMM                  # multiple of GMM
    offs = np.concatenate([[0], np.cumsum(Ks)]).astype(np.int64)
    TOT = int(offs[-1])

    counts = np.bincount(dest_global, minlength=N)
    starts = np.zeros(N, np.int64)
    np.cumsum(counts[:-1], out=starts[1:])
    k_idx = np.arange(len(rs)) - starts[dest_global]

    r = rank_of[core, loc]
    wi = r // 128
    pi = r % 128
    slot = offs[wi] + k_idx

    colf = np.full((NCORES, 128, TOT), N, np.int64)     # N -> zero row
    valf = np.zeros((NCORES, 128, TOT), np.float16)
    mask = np.zeros((NCORES, 128, TOT), np.float16)
    colf[core, pi, slot] = cs
    valf[core, pi, slot] = vs
    mask[core, pi, slot] = 1.0

    # zero-degree dests: one dummy slot with mask=1, val=0 -> den=1, num=0
    for c in range(NCORES):
        zr = np.nonzero(degs_sorted[c] == 0)[0]
        if len(zr):
            mask[c, zr % 128, offs[zr // 128]] = 1.0

    return perm, Ks, offs, TOT, colf, valf, mask


def _build_he(h16ext, colf_c, mask_c, Ks, offs, TOT):
    """he_in for one core: per window [128, 65, K] blocks, flattened."""
    g = h16ext[colf_c]                                  # [128, TOT, 64]
    he = np.empty((128, 65 * TOT), np.float16)
    for w in range(NW):
        a, b = int(offs[w]), int(offs[w + 1])
        blk = np.empty((128, 65, b - a), np.float16)
        blk[:, 0:D, :] = np.swapaxes(g[:, a:b, :], 1, 2)
        blk[:, D, :] = mask_c[:, a:b]
        he[:, 65 * a:65 * b] = blk.reshape(128, -1)
    return he


def kernel(x, kernel, edge_vals, rows, cols, nodes_num):
    assert int(nodes_num) == N and x.shape == (N, F) and kernel.shape == (F, D)
    x = np.asarray(x, dtype=np.float32)
    kernel = np.ascontiguousarray(np.asarray(kernel, dtype=np.float32))
    edge_vals = np.asarray(edge_vals, dtype=np.float32)
    rows = np.asarray(rows)
    cols = np.asarray(cols)

    for name, fn in (("stats", _build_stats), ("h", _build_h)):
        if name not in _cache:
            _cache[name] = fn()

    x16 = x.astype(np.float16)
    xT_maps = [np.ascontiguousarray(x16[c * NPC:(c + 1) * NPC, :].T)
               for c in range(NCORES)]

    # ---- program A1: partial BN stats ----
    res_s = run_bass_kernel_spmd(
        _cache["stats"], [{"xT": xT_maps[c]} for c in range(NCORES)],
        core_ids=list(range(NCORES)))
    # host CONCATENATES (indexing only); the sum happens on-device in A2
    stats_all = np.ascontiguousarray(np.concatenate(
        [res_s.results[c]["st"][:, None, :] for c in range(NCORES)],
        axis=1).reshape(F, NCORES * 2))

    # ---- program A2: h = BN(x) @ W ----
    res_h = run_bass_kernel_spmd(
        _cache["h"],
        [{"xT": xT_maps[c], "w_in": kernel, "stats_in": stats_all}
         for c in range(NCORES)],
        core_ids=list(range(NCORES)))
    h16 = np.concatenate(
        [res_h.results[c]["hT"].T for c in range(NCORES)], axis=0)  # [N, 64]

    # ---- host: edge layout + gather (indexing only) ----
    perm, Ks, offs, TOT, colf, valf, mask = _edge_layout(rows, cols, edge_vals)
    key = ("edge",) + tuple(int(k) for k in Ks)
    if key not in _cache:
        _cache[key] = _build_edge(Ks)
    nc_e = _cache[key]

    h16ext = np.vstack([h16, np.zeros((1, D), np.float16)])
    ident = np.eye(128, dtype=np.float16)
    in_maps_e = []
    for c in range(NCORES):
        in_maps_e.append({
            "he_in": _build_he(h16ext, colf[c], mask[c], Ks, offs, TOT),
            "val_in": np.ascontiguousarray(valf[c]),
            "ident_in": ident,
        })
    res_e = run_bass_kernel_spmd(nc_e, in_maps_e, core_ids=list(range(NCORES)))

    out = np.empty((N, D), np.float32)
    for c in range(NCORES):
        o = res_e.results[c]["out"][:NPC].astype(np.float32)  # rank order
        out[c * NPC + perm[c], :] = o
    return out


# revision 15
# speedup vs baseline: 2.3902x; 1.0164x over previous
"""GCN layer (BN -> dense -> sparse softmax -> gather/scatter -> tanh) on 8
Trainium2 NeuronCores.

Strategy (three small device programs, h-space aggregation, identity scatter):

 Program A1: per core, load its 12500-node slab of x^T and emit BN partial
 sums [128, 2] (sum, sum of squares). No collective — the cross-core
 reduction rides the host round-trip that the edge gather needs anyway
 (host only CONCATENATES the 8 partial tiles; all arithmetic on device).

 Program A2: per core, read all 8 partial-stat tiles, finish mean/rstd,
 fold BN into the projection (W' = rstd*W, b' = -mean*rstd @ W), and emit
 h = BN(x) @ W as fp16 [64, 12500] for its slab.

 Host (indexing only): edges are sharded by destination core. Within a
 core, destinations are sorted by degree and packed 128 per window, one
 PARTITION ROW per destination; window w holds K_w = max degree in the
 window chunks. The k-th edge of a destination sits at chunk k of its
 partition row, so the scatter matrix of every chunk is the IDENTITY.
 The host gathers h[col] for each edge slot into he_w [128, 65, K_w]
 (feature-major, chunk axis innermost; channel 64 is the edge mask that
 yields the softmax denominator).

 Program B: windows are processed in groups of 4 (one ~1 MB input DMA and
 one output DMA per group, alternating across two queues). Per window:
 ONE batched vector multiply by exp(val) (broadcast along the feature
 axis; innermost contiguous so the DVE packs 2 fp16/cycle), then K/4
 identity-stationary matmuls accumulate 4 chunks each into PSUM
 [128, 65, 4]; a vector reduce folds the 4 lanes; reciprocal is batched
 per group; out = tanh(rec * A[:, 0:64]) fused on the scalar engine.

 Softmax needs no max subtraction (edge_vals are uniform [0,1)).
 Zero-degree destinations get one dummy mask=1,val=0 slot -> den=1,
 num=0 -> tanh(0)=0, matching segment_sum semantics.
"""
import sys

sys.path.insert(0, "/opt/trn_rl_repo")

import numpy as np
from contextlib import ExitStack

import concourse.bass as bass
import concourse.bacc as bacc
import concourse.mybir as mybir
import concourse.tile as tile
from concourse.bass_utils import run_bass_kernel_spmd

# problem constants
N = 100000
E = 1600000
F = 128
D = 64
BN_EPS = 1e-3
NCORES = 8
NPC = N // NCORES            # 12500 destination nodes per core
NW = (NPC + 127) // 128      # 98 windows per core (last window 84 dests)
GW = 4                       # windows per DMA group
GMM = 4                      # chunks folded per matmul (PSUM [128, 65, GMM])

f16, f32 = mybir.dt.float16, mybir.dt.float32

_cache: dict = {}            # name -> program


def _round_up_size(size):
    for valid in (32, 64, 128):
        if valid >= size:
            return valid
    raise AssertionError(size)


def _mm(nc, out, lhsT, rhs, start, stop, ldw):
    """matmul with explicit control of the ldweights flag."""
    te = nc.tensor
    ifmap_ap = te.lower_ap(rhs.opt({0}), opt=False)
    weights_ap = te.lower_ap(lhsT.opt({0}), opt=False, for_matmul_weights=True)
    out_ap = te.lower_ap(out)
    tile_size = (_round_up_size(rhs.partition_size()),
                 _round_up_size(out.partition_size()))
    inst = mybir.InstMatmult(
        name=te.bass.get_next_instruction_name(),
        replication_resolution=0,
        replication_shift_amnt=0,
        replication_num_rows=0,
        start_tensor_calc=start,
        stop_tensor_calc=stop,
        ins=[ifmap_ap, weights_ap],
        outs=[out_ap],
        perf_mode=None,
        is_transpose=None,
        ifmap_quant_offset=None,
        weights_quant_offset=None,
        bass_skip_group_check=False,
        tile_position=(0, 0),
        tile_size=tile_size,
        ldweights=ldw,
    )
    return te.add_instruction(inst)


def _build_stats():
    """Program A1: per-core BN partial sums [128, 2] = [sum, sumsq]."""
    nc = bacc.Bacc(None, target_bir_lowering=False)
    xT = nc.declare_dram_parameter("xT", [F, NPC], f16, isOutput=False)
    st_out = nc.declare_dram_parameter("st", [F, 2], f32, isOutput=True)

    with tile.TileContext(nc) as tc:
        with ExitStack() as ctx:
            sb = ctx.enter_context(tc.tile_pool(name="sb", bufs=1))

            xts = sb.tile([F, NPC], f16)
            half = NPC // 2
            nc.sync.dma_start(out=xts[:, 0:half], in_=xT[:, 0:half])
            nc.gpsimd.dma_start(out=xts[:, half:], in_=xT[:, half:])

            stats = sb.tile([F, 2], f32)
            # sum on vector (full length), sumsq on scalar (full length) —
            # the two engines run concurrently
            nc.vector.tensor_reduce(
                out=stats[:, 0:1], in_=xts[:], axis=mybir.AxisListType.X,
                op=mybir.AluOpType.add)
            sq_trash = sb.tile([F, NPC], f16)
            nc.scalar.activation(
                out=sq_trash[:], in_=xts[:],
                func=mybir.ActivationFunctionType.Square,
                accum_out=stats[:, 1:2])
            nc.sync.dma_start(out=st_out[:], in_=stats[:])

    nc.finalize()
    return nc


def _build_h():
    """Program A2: finish BN from all-core partials, project h = BN(x)@W."""
    nc = bacc.Bacc(None, target_bir_lowering=False)
    xT = nc.declare_dram_parameter("xT", [F, NPC], f16, isOutput=False)
    w_in = nc.declare_dram_parameter("w_in", [F, D], f32, isOutput=False)
    stats_in = nc.declare_dram_parameter("stats_in", [F, NCORES * 2], f32,
                                         isOutput=False)
    hT = nc.declare_dram_parameter("hT", [D, NPC], f16, isOutput=True)

    NT = 500
    NCHUNK = NPC // NT

    with tile.TileContext(nc) as tc:
        with ExitStack() as ctx:
            sb = ctx.enter_context(tc.tile_pool(name="sb", bufs=1))
            pp = ctx.enter_context(tc.tile_pool(name="pp", bufs=1, space="PSUM"))

            stp = sb.tile([F, NCORES, 2], f32)
            nc.sync.dma_start(out=stp[:], in_=stats_in[:])
            red = sb.tile([F, 2], f32)
            # sum over the 8 cores: view [F, 2, 8] (stride trick) reduce X
            nc.vector.tensor_reduce(
                out=red[:, :, None],
                in_=stp[:].rearrange("p c s -> p s c"),
                axis=mybir.AxisListType.X, op=mybir.AluOpType.add)

            w_sb = sb.tile([F, D], f32)
            nc.gpsimd.dma_start(out=w_sb[:], in_=w_in[:])
            xts = sb.tile([F, NPC], f16)
            half = NPC // 2
            nc.sync.dma_start(out=xts[:, 0:half], in_=xT[:, 0:half])
            nc.gpsimd.dma_start(out=xts[:, half:], in_=xT[:, half:])

            mean = sb.tile([F, 1], f32)
            nc.vector.tensor_scalar_mul(out=mean[:], in0=red[:, 0:1],
                                        scalar1=1.0 / N)
            ex2 = sb.tile([F, 1], f32)
            nc.vector.tensor_scalar_mul(out=ex2[:], in0=red[:, 1:2],
                                        scalar1=1.0 / N)
            msq = sb.tile([F, 1], f32)
            nc.vector.tensor_tensor(out=msq[:], in0=mean[:], in1=mean[:],
                                    op=mybir.AluOpType.mult)
            varep = sb.tile([F, 1], f32)
            nc.vector.tensor_tensor(out=varep[:], in0=ex2[:], in1=msq[:],
                                    op=mybir.AluOpType.subtract)
            nc.vector.tensor_scalar_add(out=varep[:], in0=varep[:],
                                        scalar1=BN_EPS)
            sdev = sb.tile([F, 1], f32)
            nc.scalar.activation(out=sdev[:], in_=varep[:],
                                 func=mybir.ActivationFunctionType.Sqrt)
            rstd = sb.tile([F, 1], f32)
            nc.vector.reciprocal(out=rstd[:], in_=sdev[:])

            w16 = sb.tile([F, D], f16)
            nc.vector.tensor_scalar(out=w16[:], in0=w_sb[:],
                                    scalar1=rstd[:, 0:1], scalar2=None,
                                    op0=mybir.AluOpType.mult)
            nmr = sb.tile([F, 1], f32)
            nc.vector.tensor_tensor(out=nmr[:], in0=mean[:], in1=rstd[:],
                                    op=mybir.AluOpType.mult)
            nmr16 = sb.tile([F, 1], f16)
            nc.vector.tensor_scalar_mul(out=nmr16[:], in0=nmr[:], scalar1=-1.0)

            b_ps = pp.tile([D, 1], f32, tag="b")
            nc.tensor.matmul(out=b_ps[:], lhsT=w16[:], rhs=nmr16[:],
                             start=True, stop=True)
            bvec = sb.tile([D, 1], f32)
            nc.vector.tensor_copy(out=bvec[:], in_=b_ps[:])

            hT16 = sb.tile([D, NPC], f16)
            nc.tensor.ldweights(w16[:])
            for t in range(NCHUNK):
                s = t * NT
                hps = pp.tile([D, NT], f32, tag="h", bufs=4)
                _mm(nc, out=hps[:], lhsT=w16[:], rhs=xts[:, s:s + NT],
                    start=True, stop=True, ldw=False)
                nc.scalar.activation(out=hT16[:, s:s + NT], in_=hps[:],
                                     func=mybir.ActivationFunctionType.Identity,
                                     bias=bvec[:, 0:1])
                if t % 5 == 4:
                    q = nc.sync if (t // 5) % 2 == 0 else nc.gpsimd
                    q.dma_start(out=hT[:, s - 4 * NT:s + NT],
                                in_=hT16[:, s - 4 * NT:s + NT])

    nc.finalize()
    return nc


def _build_edge(Ks):
    """Program B: per-window scaled accumulate + softmax-normalize + tanh."""
    Ks = list(Ks)
    offs = np.concatenate([[0], np.cumsum(Ks)]).astype(np.int64)
    TOT = int(offs[-1])

    nc = bacc.Bacc(None, target_bir_lowering=False)

    he_in = nc.declare_dram_parameter("he_in", [128, 65 * TOT], f16,
                                      isOutput=False)
    val_in = nc.declare_dram_parameter("val_in", [128, TOT], f16,
                                       isOutput=False)
    ident_in = nc.declare_dram_parameter("ident_in", [128, 128], f16,
                                         isOutput=False)
    out_p = nc.declare_dram_parameter("out", [NW * 128, D], f16, isOutput=True)

    queues = [nc.sync, nc.gpsimd, nc.scalar]
    groups = [list(range(g0, min(g0 + GW, NW))) for g0 in range(0, NW, GW)]

    with tile.TileContext(nc) as tc:
        with ExitStack() as ctx:
            sb = ctx.enter_context(tc.tile_pool(name="sb", bufs=1))
            pp = ctx.enter_context(tc.tile_pool(name="pp", bufs=1, space="PSUM"))

            ident_sb = sb.tile([128, 128], f16)
            nc.sync.dma_start(out=ident_sb[:], in_=ident_in[:])
            val_sb = sb.tile([128, TOT], f16)
            nc.sync.dma_start(out=val_sb[:], in_=val_in[:])
            exp_sb = sb.tile([128, TOT], f16)
            nc.scalar.activation(out=exp_sb[:], in_=val_sb[:],
                                 func=mybir.ActivationFunctionType.Exp)

            nc.tensor.ldweights(ident_sb[:])
            for gi, gwin in enumerate(groups):
                q = queues[gi % 3]
                qo = queues[(gi + 1) % 3]
                a = int(offs[gwin[0]])
                b = int(offs[gwin[-1] + 1])
                GK = b - a
                ng = len(gwin)
                he_g = sb.tile([128, 65 * GK], f16, tag="he", bufs=3)
                q.dma_start(out=he_g[:], in_=he_in[:, 65 * a:65 * b])
                og = sb.tile([128, ng, D], f16, tag="og", bufs=3)
                Afg = sb.tile([128, ng, 65], f32, tag="Af", bufs=3)
                for wi, w in enumerate(gwin):
                    K = Ks[w]
                    ca = int(offs[w]) - a
                    he_w = he_g[:, 65 * ca:65 * (ca + K)].rearrange(
                        "p (f k) -> p f k", k=K)
                    hp_w = sb.tile([128, 65, K], f16, tag="hp", bufs=4)
                    nc.vector.tensor_tensor(
                        out=hp_w[:], in0=he_w,
                        in1=exp_sb[:, None, a + ca:a + ca + K]
                            .to_broadcast([128, 65, K]),
                        op=mybir.AluOpType.mult)
                    A = pp.tile([128, 65, GMM], f32, tag="A", bufs=4)
                    nmm = K // GMM
                    for j in range(nmm):
                        _mm(nc, out=A[:], lhsT=ident_sb[:],
                            rhs=hp_w[:, :, GMM * j:GMM * (j + 1)],
                            start=(j == 0), stop=(j == nmm - 1),
                            ldw=False)
                    nc.vector.tensor_reduce(
                        out=Afg[:, wi, :, None], in_=A[:],
                        axis=mybir.AxisListType.X, op=mybir.AluOpType.add)
                rec_g = sb.tile([128, ng], f32, tag="rec", bufs=3)
                nc.vector.reciprocal(out=rec_g[:],
                                     in_=Afg[:, :, 64])
                for wi, w in enumerate(gwin):
                    nc.scalar.activation(out=og[:, wi, :],
                                         in_=Afg[:, wi, 0:D],
                                         func=mybir.ActivationFunctionType.Tanh,
                                         scale=rec_g[:, wi:wi + 1])
                g0 = gwin[0]
                qo.dma_start(
                    out=out_p[g0 * 128:(g0 + ng) * 128, :]
                        .rearrange("(w p) f -> p w f", w=ng),
                    in_=og[:])

    nc.finalize()
    return nc


def _edge_layout(rows, cols, edge_vals):
    """Degree-sorted identity layout. Host does indexing only."""
    order = np.argsort(rows, kind="stable")
    rs = rows[order].astype(np.int64)
    cs = cols[order].astype(np.int64)
    vs = edge_vals[order].astype(np.float16)

    core = rs // NPC
    loc = rs % NPC
    dest_global = core * NPC + loc

    deg = np.bincount(dest_global, minlength=N).reshape(NCORES, NPC)
    perm = np.argsort(-deg, axis=1, kind="stable")      # rank -> dest id
    rank_of = np.empty_like(perm)
    rows_idx = np.arange(NPC)
    for c in range(NCORES):
        rank_of[c, perm[c]] = rows_idx
    degs_sorted = -np.sort(-deg, axis=1)

    Kc = degs_sorted[:, ::128][:, :NW]                  # [NCORES, NW]
    Ks = Kc.max(axis=0)
    Ks = np.maximum(Ks, GMM)
    Ks = ((Ks + GMM - 1) // GMM) * GMM                  # multiple of GMM
    offs = np.concatenate([[0], np.cumsum(Ks)]).astype(np.int64)
    TOT = int(offs[-1])

    counts = np.bincount(dest_global, minlength=N)
    starts = np.zeros(N, np.int64)
    np.cumsum(counts[:-1], out=starts[1:])
    k_idx = np.arange(len(rs)) - starts[dest_global]

    r = rank_of[core, loc]
    wi = r // 128
    pi = r % 128
    slot = offs[wi] + k_idx

    colf = np.full((NCORES, 128, TOT), N, np.int64)     # N -> zero row
    valf = np.zeros((NCORES, 128, TOT), np.float16)
    mask = np.zeros((NCORES, 128, TOT), np.float16)
    colf[core, pi, slot] = cs
    valf[core, pi, slot] = vs
    mask[core, pi, slot] = 1.0

    # zero-degree dests: one dummy slot with mask=1, val=0 -> den=1, num=0
    for c in range(NCORES):
        zr = np.nonzero(degs_sorted[c] == 0)[0]
        if len(zr):
            mask[c, zr % 128, offs[zr // 128]] = 1.0

    return perm, Ks, offs, TOT, colf, valf, mask


def _build_he(h16ext, colf_c, mask_c, Ks, offs, TOT):
    """he_in for one core: per window [128, 65, K] blocks, flattened."""
    g = h16ext[colf_c]                                  # [128, TOT, 64]
    he = np.empty((128, 65 * TOT), np.float16)
    for w in range(NW):
        a, b = int(offs[w]), int(offs[w + 1])
        blk = np.empty((128, 65, b - a), np.float16)
        blk[:, 0:D, :] = np.swapaxes(g[:, a:b, :], 1, 2)
        blk[:, D, :] = mask_c[:, a:b]
        he[:, 65 * a:65 * b] = blk.reshape(128, -1)
    return he


def kernel(x, kernel, edge_vals, rows, cols, nodes_num):
    assert int(nodes_num) == N and x.shape == (N, F) and kernel.shape == (F, D)
    x = np.asarray(x, dtype=np.float32)
    kernel = np.ascontiguousarray(np.asarray(kernel, dtype=np.float32))
    edge_vals = np.asarray(edge_vals, dtype=np.float32)
    rows = np.asarray(rows)
    cols = np.asarray(cols)

    for name, fn in (("stats", _build_stats), ("h", _build_h)):
        if name not in _cache:
            _cache[name] = fn()

    x16 = x.astype(np.float16)
    xT_maps = [np.ascontiguousarray(x16[c * NPC:(c + 1) * NPC, :].T)
               for c in range(NCORES)]

    # ---- program A1: partial BN stats ----
    res_s = run_bass_kernel_spmd(
        _cache["stats"], [{"xT": xT_maps[c]} for c in range(NCORES)],
        core_ids=list(range(NCORES)))
    # host CONCATENATES (indexing only); the sum happens on-device in A2
    stats_all = np.ascontiguousarray(np.concatenate(
        [res_s.results[c]["st"][:, None, :] for c in range(NCORES)],
        axis=1).reshape(F, NCORES * 2))

    # ---- program A2: h = BN(x) @ W ----
    res_h = run_bass_kernel_spmd(
        _cache["h"],
        [{"xT": xT_maps[c], "w_in": kernel, "stats_in": stats_all}
         for c in range(NCORES)],
        core_ids=list(range(NCORES)))
    h16 = np.concatenate(
        [res_h.results[c]["hT"].T for c in range(NCORES)], axis=0)  # [N, 64]

    # ---- host: edge layout + gather (indexing only) ----
    perm, Ks, offs, TOT, colf, valf, mask = _edge_layout(rows, cols, edge_vals)
    key = ("edge",) + tuple(int(k) for k in Ks)
    if key not in _cache:
        _cache[key] = _build_edge(Ks)
    nc_e = _cache[key]

    h16ext = np.vstack([h16, np.zeros((1, D), np.float16)])
    ident = np.eye(128, dtype=np.float16)
    in_maps_e = []
    for c in range(NCORES):
        in_maps_e.append({
            "he_in": _build_he(h16ext, colf[c], mask[c], Ks, offs, TOT),
            "val_in": np.ascontiguousarray(valf[c]),
            "ident_in": ident,
        })
    res_e = run_bass_kernel_spmd(nc_e, in_maps_e, core_ids=list(range(NCORES)))

    out = np.empty((N, D), np.float32)
    for c in range(NCORES):
        o = res_e.results[c]["out"][:NPC].astype(np.float32)  # rank order
        out[c * NPC + perm[c], :] = o
    return out
